# revision 7
# baseline (speedup 1.0000x reference)
"""Causal depthwise conv1d (B=8, L=4096, C=1024, K=7) on 8 Trainium2 cores.

Strategy:
  - Pure data parallel: one batch element per NeuronCore.
  - Host casts x to fp16 (rel err ~3e-4 end to end) and builds per-channel-
    group diagonal weight matrices so the depthwise conv maps onto the
    TensorEngine as accumulating diagonal matmuls (PSUM fp32 accumulation).
  - Device layout: channels-on-partitions via DMA xbar transpose (fp16).
    The 7 taps are split across engines to balance busy time:
      * taps 2..6  -> PE diagonal matmuls into PSUM
      * tap 1+bias -> ScalarE activation (per-partition scale & bias)
      * tap 0      -> VectorE scalar_tensor_tensor fused with the PSUM drain
      * merge      -> GpSimd tensor_add
    The output [C, L] -> [L, C] transpose runs on PE (128x128 fp16 transposes
    into PSUM) with DVE/ACT copies, then contiguous fp16 stores.
  - Host casts fp16 result back to fp32.
"""

import os
import sys

import numpy as np

if "/opt/trn_rl_repo" not in sys.path:
    sys.path.append("/opt/trn_rl_repo")

B, L, C, K = 8, 4096, 1024, 7
G = C // 128            # channel groups of 128 partitions
PAD = 16                # left zero pad (>= K-1, 32B aligned for xbar dest)
U = 1024                # unit: free-dim span per PSUM accumulation tile
NU = L // U             # units per group
NCHUNK = 512            # matmul free-dim chunk (one PSUM fp32 bank)
PE_TAPS = range(2, K)   # taps on the TensorEngine
ACT_TAP = 1             # tap on ScalarE (carries the bias)
DVE_TAP = 0             # tap on VectorE (fused with PSUM drain)

_CACHE: dict = {}
LAST_RESULTS = None     # BassKernelResults of the most recent run (for test.py)


def _build_device_program():
    import concourse.bacc as bacc
    import concourse.mybir as mybir
    from concourse.tile import TileContext

    fp16 = mybir.dt.float16
    fp32 = mybir.dt.float32
    Identity = mybir.ActivationFunctionType.Identity
    mult = mybir.AluOpType.mult
    add = mybir.AluOpType.add

    nc = bacc.Bacc(
        "TRN2",
        target_bir_lowering=False,
        debug=False,
        enable_asserts=False,
        num_devices=8,
    )

    x16 = nc.dram_tensor("x16", [L, C], fp16, kind="ExternalInput").ap()
    wd = nc.dram_tensor("wd", [128, G * K * 128], fp16, kind="ExternalInput").ap()
    wv = nc.dram_tensor("wv", [128, G * K], fp32, kind="ExternalInput").ap()
    bv = nc.dram_tensor("bv", [128, G], fp32, kind="ExternalInput").ap()
    idm = nc.dram_tensor("idm", [128, 128], fp16, kind="ExternalInput").ap()
    y16 = nc.dram_tensor("y16", [L, C], fp16, kind="ExternalOutput").ap()

    with TileContext(nc) as tc:
        with (
            tc.tile_pool(name="wpool", bufs=1) as wpool,
            tc.tile_pool(name="xpool", bufs=1) as xpool,
            tc.tile_pool(name="partials", bufs=3) as partials,
            tc.tile_pool(name="ypool", bufs=2) as ypool,
            tc.tile_pool(name="opool", bufs=4) as opool,
            tc.tile_pool(name="pspool", bufs=3, space="PSUM") as pspool,
            tc.tile_pool(name="ptpool", bufs=2, space="PSUM") as ptpool,
        ):
            wtile = wpool.tile([128, G * K * 128], fp16, tag="w")
            nc.sync.dma_start(wtile[:], wd[:])
            wvt = wpool.tile([128, G * K], fp32, tag="wv")
            nc.sync.dma_start(wvt[:], wv[:])
            bvt = wpool.tile([128, G], fp32, tag="bv")
            nc.sync.dma_start(bvt[:], bv[:])
            ident = wpool.tile([128, 128], fp16, tag="id")
            nc.sync.dma_start(ident[:], idm[:])

            # x transposed per group: [128 ch, PAD + L], causal zero pad.
            xts = []
            for g in range(G):
                xt = xpool.tile([128, PAD + L], fp16, tag=f"xt{g}")
                nc.vector.memset(xt[:, 0:PAD], 0.0)
                nc.sync.dma_start_transpose(
                    xt[:, PAD : PAD + L], x16[:, g * 128 : (g + 1) * 128]
                )
                xts.append(xt)

            ycl = {}
            for h in range(NU):
                for g in range(G):
                    xt = xts[g]
                    base = PAD - (K - 1) + h * U
                    ps = pspool.tile([128, U], fp32, tag="ps")
                    for j in PE_TAPS:
                        lhsT = wtile[:, (g * K + j) * 128 : (g * K + j + 1) * 128]
                        for n in range(U // NCHUNK):
                            a = base + j + n * NCHUNK
                            nc.tensor.matmul(
                                ps[:, n * NCHUNK : (n + 1) * NCHUNK],
                                lhsT,
                                xt[:, a : a + NCHUNK],
                                start=(j == PE_TAPS[0]),
                                stop=(j == PE_TAPS[-1]),
                            )
                    # tap 1 + bias on ScalarE
                    pb = partials.tile([128, U], fp16, tag="pb")
                    nc.scalar.activation(
                        pb[:],
                        xt[:, base + ACT_TAP : base + ACT_TAP + U],
                        Identity,
                        bias=bvt[:, g : g + 1],
                        scale=wvt[:, g * K + ACT_TAP : g * K + ACT_TAP + 1],
                    )
                    # tap 0 fused with PSUM drain on VectorE
                    t = partials.tile([128, U], fp16, tag="t")
                    nc.vector.scalar_tensor_tensor(
                        t[:],
                        xt[:, base + DVE_TAP : base + DVE_TAP + U],
                        wvt[:, g * K + DVE_TAP : g * K + DVE_TAP + 1],
                        ps[:],
                        mult,
                        add,
                    )
                    # merge on GpSimd
                    yt = ypool.tile([128, U], fp16, tag=f"y{g}")
                    nc.gpsimd.tensor_add(yt[:], t[:], pb[:])
                    ycl[(g, h)] = yt

                # transpose-back + store for this quarter of L
                for lb8 in range(U // 128):
                    lb = h * (U // 128) + lb8
                    pst = ptpool.tile([128, G * 128], fp16, tag="pst")
                    for g in range(G):
                        nc.tensor.transpose(
                            pst[:, g * 128 : (g + 1) * 128],
                            ycl[(g, h)][:, lb8 * 128 : (lb8 + 1) * 128],
                            ident[:],
                        )
                    ot = opool.tile([128, C], fp16, tag="ot")
                    if lb % 2 == 0:
                        nc.vector.tensor_copy(ot[:], pst[:])
                    else:
                        nc.scalar.copy(ot[:], pst[:])
                    nc.sync.dma_start(y16[lb * 128 : (lb + 1) * 128, :], ot[:])

    nc.compile()
    return nc


def _get_program():
    if "nc" not in _CACHE:
        _CACHE["nc"] = _build_device_program()
    return _CACHE["nc"]


def kernel(x, weight, bias):
    global LAST_RESULTS
    from concourse import bass_utils

    x = np.asarray(x)
    weight = np.asarray(weight)
    bias = np.asarray(bias)

    nc = _get_program()

    # Host-side prep: per-core batch shard (fp16) + replicated weights.
    w = weight[:, 0, :]  # [C, K]
    w16 = w.astype(np.float16)
    wd4 = np.zeros((G, K, 128, 128), dtype=np.float16)
    idx = np.arange(128)
    for g in range(G):
        for j in range(K):
            wd4[g, j, idx, idx] = w16[g * 128 : (g + 1) * 128, j]
    wd = np.ascontiguousarray(wd4.transpose(2, 0, 1, 3).reshape(128, G * K * 128))
    # per-partition scalar weights [p, g*K+j] (fp32)
    wv = np.ascontiguousarray(
        w.astype(np.float32).reshape(G, 128, K).transpose(1, 0, 2).reshape(128, G * K)
    )
    bv = np.ascontiguousarray(bias.astype(np.float32).reshape(G, 128).T)
    idm = np.eye(128, dtype=np.float16)

    in_maps = []
    for b in range(B):
        in_maps.append(
            {
                "x16": np.ascontiguousarray(x[b]).astype(np.float16),
                "wd": wd,
                "wv": wv,
                "bv": bv,
                "idm": idm,
            }
        )

    trace = bool(int(os.environ.get("KERNEL_TRACE", "0")))
    res = bass_utils.run_bass_kernel_spmd(
        nc, in_maps, core_ids=list(range(B)), trace=trace
    )
    LAST_RESULTS = res
    _CACHE["last_in_maps"] = in_maps

    out = np.empty((B, L, C), dtype=np.float32)
    for b in range(B):
        out[b] = res.results[b]["y16"].astype(np.float32)
    return out


# revision 23
# speedup vs baseline: 1.1043x; 1.1043x over previous
"""Causal depthwise conv1d (B=8, L=4096, C=1024, K=7) on 8 Trainium2 cores.

Strategy:
  - Pure data parallel: one batch element per NeuronCore.
  - Host casts x to fp16 (rel err ~3e-4 end to end) and builds per-channel-
    group diagonal weight matrices so the depthwise conv maps onto the
    TensorEngine as accumulating diagonal matmuls (PSUM fp32 accumulation).
  - Device layout: channels-on-partitions via DMA xbar transpose (fp16).
    The 7 taps are split across engines to balance busy time:
      * taps 2..6  -> PE diagonal matmuls into PSUM
      * tap 1+bias -> ScalarE activation (per-partition scale & bias)
      * tap 0      -> VectorE scalar_tensor_tensor fused with the PSUM drain
      * merge      -> GpSimd tensor_add
    The output [C, L] -> [L, C] transpose runs on PE (128x128 fp16 transposes
    into PSUM) with DVE/ACT copies, then contiguous fp16 stores.
  - Host casts fp16 result back to fp32.
"""

import os
import sys

import numpy as np

if "/opt/trn_rl_repo" not in sys.path:
    sys.path.append("/opt/trn_rl_repo")

B, L, C, K = 8, 4096, 1024, 7
G = C // 128            # channel groups of 128 partitions
PAD = 16                # left zero pad (>= K-1, 32B aligned for xbar dest)
U = 1024                # unit: free-dim span per PSUM accumulation tile
NU = L // U             # units per group
NCHUNK = 512            # matmul free-dim chunk (one PSUM fp32 bank)
PE_TAPS = range(2, K)   # taps on the TensorEngine
ACT_TAP = 1             # tap on ScalarE (carries the bias)
DVE_TAP = 0             # tap on VectorE (fused with PSUM drain)

_CACHE: dict = {}
LAST_RESULTS = None     # BassKernelResults of the most recent run (for test.py)


def _build_device_program():
    import concourse.bacc as bacc
    import concourse.mybir as mybir
    from concourse.tile import TileContext

    fp16 = mybir.dt.float16
    fp32 = mybir.dt.float32
    Identity = mybir.ActivationFunctionType.Identity
    mult = mybir.AluOpType.mult
    add = mybir.AluOpType.add

    nc = bacc.Bacc(
        "TRN2",
        target_bir_lowering=False,
        debug=False,
        enable_asserts=False,
        num_devices=8,
    )

    x16 = nc.dram_tensor("x16", [L, C], fp16, kind="ExternalInput").ap()
    wd = nc.dram_tensor("wd", [128, G * K * 128], fp16, kind="ExternalInput").ap()
    wv = nc.dram_tensor("wv", [128, G * K], fp32, kind="ExternalInput").ap()
    bv = nc.dram_tensor("bv", [128, G], fp32, kind="ExternalInput").ap()
    idm = nc.dram_tensor("idm", [128, 128], fp16, kind="ExternalInput").ap()
    y16 = nc.dram_tensor("y16", [L, C], fp16, kind="ExternalOutput").ap()

    with TileContext(nc) as tc:
        with (
            tc.tile_pool(name="wpool", bufs=1) as wpool,
            tc.tile_pool(name="xpool", bufs=1) as xpool,
            tc.tile_pool(name="partials", bufs=3) as partials,
            tc.tile_pool(name="ypool", bufs=2) as ypool,
            tc.tile_pool(name="opool", bufs=4) as opool,
            tc.tile_pool(name="pspool", bufs=3, space="PSUM") as pspool,
            tc.tile_pool(name="ptpool", bufs=2, space="PSUM") as ptpool,
        ):
            # Warm the ScalarE activation table at t=0 (overlaps the DMAs;
            # the first real activation would otherwise pay ~2.7us mid-path).
            warm = wpool.tile([128, 1], fp32, tag="warm")
            nc.vector.memset(warm[:], 0.0)
            nc.scalar.activation(
                warm[:], warm[:], mybir.ActivationFunctionType.Identity, bias=0.0
            )

            # x loads are chunked [U rows at a time] and ordered to match the
            # (h-outer, g-inner) consumption order, so PE starts ~3us in and
            # the loader stays ahead of compute. Small scalar tables and
            # group-0 weights go right after the first chunk.
            def load_chunk(g, h):
                nc.sync.dma_start_transpose(
                    xts[g][:, PAD + h * U : PAD + (h + 1) * U],
                    x16[h * U : (h + 1) * U, g * 128 : (g + 1) * 128],
                )

            xts = []
            for g in range(G):
                xt = xpool.tile([128, PAD + L], fp16, tag=f"xt{g}")
                nc.vector.memset(xt[:, 0:PAD], 0.0)
                xts.append(xt)

            load_chunk(0, 0)
            wvt = wpool.tile([128, G * K], fp32, tag="wv")
            nc.sync.dma_start(wvt[:], wv[:])
            bvt = wpool.tile([128, G], fp32, tag="bv")
            nc.sync.dma_start(bvt[:], bv[:])
            wtile = wpool.tile([128, G * K * 128], fp16, tag="w")
            nc.sync.dma_start(wtile[:, : K * 128], wd[:, : K * 128])
            for g in range(1, G):
                load_chunk(g, 0)
            ident = wpool.tile([128, 128], fp16, tag="id")
            nc.sync.dma_start(ident[:], idm[:])
            nc.sync.dma_start(wtile[:, K * 128 :], wd[:, K * 128 :])
            for h in range(1, NU):
                for g in range(G):
                    load_chunk(g, h)

            def out_block(lb, ycl_h):
                """Transpose one 128-row L-block back to [L, C] and store."""
                lb8 = lb % (U // 128)
                pst = ptpool.tile([128, G * 128], fp16, tag="pst")
                for g in range(G):
                    nc.tensor.transpose(
                        pst[:, g * 128 : (g + 1) * 128],
                        ycl_h[g][:, lb8 * 128 : (lb8 + 1) * 128],
                        ident[:],
                    )
                ot = opool.tile([128, C], fp16, tag="ot")
                nc.scalar.copy(ot[:], pst[:])
                nc.sync.dma_start(y16[lb * 128 : (lb + 1) * 128, :], ot[:])

            ycl = {}
            for h in range(NU):
                for g in range(G):
                    xt = xts[g]
                    base = PAD - (K - 1) + h * U
                    ps = pspool.tile([128, U], fp32, tag="ps")
                    # taps 2..6 on PE: plain PSUM accumulation, depends only
                    # on the x chunk and weights — nothing else gates PE.
                    for j in PE_TAPS:
                        lhsT = wtile[:, (g * K + j) * 128 : (g * K + j + 1) * 128]
                        for n in range(U // NCHUNK):
                            a = base + j + n * NCHUNK
                            nc.tensor.matmul(
                                ps[:, n * NCHUNK : (n + 1) * NCHUNK],
                                lhsT,
                                xt[:, a : a + NCHUNK],
                                start=(j == PE_TAPS[0]),
                                stop=(j == PE_TAPS[-1]),
                            )
                    # tap 0 on VectorE (4x tensor_scalar, SBUF out)
                    u0 = partials.tile([128, U], fp16, tag="u0")
                    nc.vector.tensor_scalar_mul(
                        u0[:],
                        xt[:, base + DVE_TAP : base + DVE_TAP + U],
                        wvt[:, g * K + DVE_TAP : g * K + DVE_TAP + 1],
                    )
                    # tap 1 on GpSimd
                    pb = partials.tile([128, U], fp16, tag="pb")
                    nc.gpsimd.tensor_scalar_mul(
                        pb[:],
                        xt[:, base + ACT_TAP : base + ACT_TAP + U],
                        wvt[:, g * K + ACT_TAP : g * K + ACT_TAP + 1],
                    )
                    # drain + bias on ScalarE (after PE's last tap)
                    yd = partials.tile([128, U], fp16, tag="yd")
                    nc.scalar.activation(
                        yd[:], ps[:], Identity, bias=bvt[:, g : g + 1], scale=1.0
                    )
                    # merges on VectorE (2x fp16 tensor_tensor)
                    v = partials.tile([128, U], fp16, tag="v")
                    nc.vector.tensor_add(v[:], u0[:], pb[:])
                    yt = ypool.tile([128, U], fp16, tag=f"y{g}")
                    nc.vector.tensor_add(yt[:], v[:], yd[:])
                    ycl[(g, h)] = yt

                    # interleave previous quarter's transpose+store with
                    # this quarter's conv so PE never sits at a barrier
                    if h > 0:
                        out_block(
                            (h - 1) * (U // 128) + g,
                            [ycl[(gg, h - 1)] for gg in range(G)],
                        )

            for lb8 in range(U // 128):
                out_block(
                    (NU - 1) * (U // 128) + lb8,
                    [ycl[(gg, NU - 1)] for gg in range(G)],
                )

    nc.compile()
    return nc


def _get_program():
    if "nc" not in _CACHE:
        _CACHE["nc"] = _build_device_program()
    return _CACHE["nc"]


def kernel(x, weight, bias):
    global LAST_RESULTS
    from concourse import bass_utils

    x = np.asarray(x)
    weight = np.asarray(weight)
    bias = np.asarray(bias)

    nc = _get_program()

    # Host-side prep: per-core batch shard (fp16) + replicated weights.
    w = weight[:, 0, :]  # [C, K]
    w16 = w.astype(np.float16)
    wd4 = np.zeros((G, K, 128, 128), dtype=np.float16)
    idx = np.arange(128)
    for g in range(G):
        for j in range(K):
            wd4[g, j, idx, idx] = w16[g * 128 : (g + 1) * 128, j]
    wd = np.ascontiguousarray(wd4.transpose(2, 0, 1, 3).reshape(128, G * K * 128))
    # per-partition scalar weights [p, g*K+j] (fp32)
    wv = np.ascontiguousarray(
        w.astype(np.float32).reshape(G, 128, K).transpose(1, 0, 2).reshape(128, G * K)
    )
    bv = np.ascontiguousarray(bias.astype(np.float32).reshape(G, 128).T)
    idm = np.eye(128, dtype=np.float16)

    in_maps = []
    for b in range(B):
        in_maps.append(
            {
                "x16": np.ascontiguousarray(x[b]).astype(np.float16),
                "wd": wd,
                "wv": wv,
                "bv": bv,
                "idm": idm,
            }
        )

    trace = bool(int(os.environ.get("KERNEL_TRACE", "0")))
    res = bass_utils.run_bass_kernel_spmd(
        nc, in_maps, core_ids=list(range(B)), trace=trace
    )
    LAST_RESULTS = res
    _CACHE["last_in_maps"] = in_maps

    out = np.empty((B, L, C), dtype=np.float32)
    for b in range(B):
        out[b] = res.results[b]["y16"].astype(np.float32)
    return out


# revision 29
# speedup vs baseline: 1.1959x; 1.0829x over previous
"""Causal depthwise conv1d (B=8, L=4096, C=1024, K=7) on 8 Trainium2 cores.

Strategy:
  - Pure data parallel: one batch element per NeuronCore.
  - Host casts x to fp16 (rel err ~3e-4 end to end) and builds per-channel-
    group diagonal weight matrices so the depthwise conv maps onto the
    TensorEngine as accumulating diagonal matmuls (PSUM fp32 accumulation).
  - Device layout: channels-on-partitions via DMA xbar transpose (fp16).
    The 7 taps are split across engines to balance busy time:
      * taps 2..6  -> PE diagonal matmuls into PSUM
      * tap 1+bias -> ScalarE activation (per-partition scale & bias)
      * tap 0      -> VectorE scalar_tensor_tensor fused with the PSUM drain
      * merge      -> GpSimd tensor_add
    The output [C, L] -> [L, C] transpose runs on PE (128x128 fp16 transposes
    into PSUM) with DVE/ACT copies, then contiguous fp16 stores.
  - Host casts fp16 result back to fp32.
"""

import os
import sys

import numpy as np

if "/opt/trn_rl_repo" not in sys.path:
    sys.path.append("/opt/trn_rl_repo")

B, L, C, K = 8, 4096, 1024, 7
G = C // 128            # channel groups of 128 partitions
PAD = 16                # left zero pad (>= K-1, 32B aligned for xbar dest)
U = 1024                # unit: free-dim span per PSUM accumulation tile
NU = L // U             # units per group
NCHUNK = 512            # matmul free-dim chunk (one PSUM fp32 bank)
PE_TAPS = range(3, K)   # taps on the TensorEngine
ACT_TAP = 1             # tap on GpSimd
DVE_TAP = 0             # tap on VectorE (4x tensor_scalar)
DVE_TAP2 = 2            # second tap on VectorE

_CACHE: dict = {}
LAST_RESULTS = None     # BassKernelResults of the most recent run (for test.py)


def _build_device_program():
    import concourse.bacc as bacc
    import concourse.mybir as mybir
    from concourse.tile import TileContext

    fp16 = mybir.dt.float16
    fp32 = mybir.dt.float32
    Identity = mybir.ActivationFunctionType.Identity
    mult = mybir.AluOpType.mult
    add = mybir.AluOpType.add

    nc = bacc.Bacc(
        "TRN2",
        target_bir_lowering=False,
        debug=False,
        enable_asserts=False,
        num_devices=8,
    )

    x16 = nc.dram_tensor("x16", [L, C], fp16, kind="ExternalInput").ap()
    wd = nc.dram_tensor("wd", [128, G * K * 128], fp16, kind="ExternalInput").ap()
    wv = nc.dram_tensor("wv", [128, G * K], fp32, kind="ExternalInput").ap()
    bv = nc.dram_tensor("bv", [128, G], fp32, kind="ExternalInput").ap()
    idm = nc.dram_tensor("idm", [128, 128], fp16, kind="ExternalInput").ap()
    y16 = nc.dram_tensor("y16", [L, C], fp16, kind="ExternalOutput").ap()

    with TileContext(nc) as tc:
        with (
            tc.tile_pool(name="wpool", bufs=1) as wpool,
            tc.tile_pool(name="xpool", bufs=1) as xpool,
            tc.tile_pool(name="partials", bufs=3) as partials,
            tc.tile_pool(name="ypool", bufs=2) as ypool,
            tc.tile_pool(name="opool", bufs=4) as opool,
            tc.tile_pool(name="pspool", bufs=3, space="PSUM") as pspool,
            tc.tile_pool(name="ptpool", bufs=2, space="PSUM") as ptpool,
        ):
            # Warm the ScalarE activation table at t=0 (overlaps the DMAs;
            # the first real activation would otherwise pay ~2.7us mid-path).
            warm = wpool.tile([128, 1], fp32, tag="warm")
            nc.vector.memset(warm[:], 0.0)
            nc.scalar.activation(
                warm[:], warm[:], mybir.ActivationFunctionType.Identity, bias=0.0
            )

            # x loads are chunked [U rows at a time] and ordered to match the
            # (h-outer, g-inner) consumption order, so PE starts ~3us in and
            # the loader stays ahead of compute. Small scalar tables and
            # group-0 weights go right after the first chunk.
            def load_chunk(g, h):
                nc.sync.dma_start_transpose(
                    xts[g][:, PAD + h * U : PAD + (h + 1) * U],
                    x16[h * U : (h + 1) * U, g * 128 : (g + 1) * 128],
                )

            xts = []
            for g in range(G):
                xt = xpool.tile([128, PAD + L], fp16, tag=f"xt{g}")
                nc.vector.memset(xt[:, 0:PAD], 0.0)
                xts.append(xt)

            load_chunk(0, 0)
            wvt = wpool.tile([128, G * K], fp32, tag="wv")
            nc.sync.dma_start(wvt[:], wv[:])
            bvt = wpool.tile([128, G], fp32, tag="bv")
            nc.sync.dma_start(bvt[:], bv[:])
            wtile = wpool.tile([128, G * K * 128], fp16, tag="w")
            nc.sync.dma_start(wtile[:, : K * 128], wd[:, : K * 128])
            for g in range(1, G):
                load_chunk(g, 0)
            ident = wpool.tile([128, 128], fp16, tag="id")
            nc.sync.dma_start(ident[:], idm[:])
            nc.sync.dma_start(wtile[:, K * 128 :], wd[:, K * 128 :])
            for h in range(1, NU):
                for g in range(G):
                    load_chunk(g, h)

            def out_block(lb, ycl_h):
                """Transpose one 128-row L-block back to [L, C] and store."""
                lb8 = lb % (U // 128)
                pst = ptpool.tile([128, G * 128], fp16, tag="pst")
                for g in range(G):
                    nc.tensor.transpose(
                        pst[:, g * 128 : (g + 1) * 128],
                        ycl_h[g][:, lb8 * 128 : (lb8 + 1) * 128],
                        ident[:],
                    )
                ot = opool.tile([128, C], fp16, tag="ot")
                nc.scalar.copy(ot[:], pst[:])
                nc.sync.dma_start(y16[lb * 128 : (lb + 1) * 128, :], ot[:])

            ycl = {}
            for h in range(NU):
                for g in range(G):
                    xt = xts[g]
                    base = PAD - (K - 1) + h * U
                    ps = pspool.tile([128, U], fp32, tag="ps")
                    # taps 2..6 on PE: plain PSUM accumulation, depends only
                    # on the x chunk and weights — nothing else gates PE.
                    for j in PE_TAPS:
                        lhsT = wtile[:, (g * K + j) * 128 : (g * K + j + 1) * 128]
                        for n in range(U // NCHUNK):
                            a = base + j + n * NCHUNK
                            nc.tensor.matmul(
                                ps[:, n * NCHUNK : (n + 1) * NCHUNK],
                                lhsT,
                                xt[:, a : a + NCHUNK],
                                start=(j == PE_TAPS[0]),
                                stop=(j == PE_TAPS[-1]),
                            )
                    # tap 0 on VectorE (4x tensor_scalar, SBUF out)
                    u0 = partials.tile([128, U], fp16, tag="u0")
                    nc.vector.tensor_scalar_mul(
                        u0[:],
                        xt[:, base + DVE_TAP : base + DVE_TAP + U],
                        wvt[:, g * K + DVE_TAP : g * K + DVE_TAP + 1],
                    )
                    # tap 1 on GpSimd
                    pb = partials.tile([128, U], fp16, tag="pb")
                    nc.gpsimd.tensor_scalar_mul(
                        pb[:],
                        xt[:, base + ACT_TAP : base + ACT_TAP + U],
                        wvt[:, g * K + ACT_TAP : g * K + ACT_TAP + 1],
                    )
                    # second VectorE tap
                    u2 = partials.tile([128, U], fp16, tag="u2")
                    nc.vector.tensor_scalar_mul(
                        u2[:],
                        xt[:, base + DVE_TAP2 : base + DVE_TAP2 + U],
                        wvt[:, g * K + DVE_TAP2 : g * K + DVE_TAP2 + 1],
                    )
                    # drain + bias on ScalarE (after PE's last tap)
                    yd = partials.tile([128, U], fp16, tag="yd")
                    nc.scalar.activation(
                        yd[:], ps[:], Identity, bias=bvt[:, g : g + 1], scale=1.0
                    )
                    # merges on VectorE (2x fp16 tensor_tensor)
                    v = partials.tile([128, U], fp16, tag="v")
                    nc.vector.tensor_add(v[:], u0[:], pb[:])
                    v2 = partials.tile([128, U], fp16, tag="v2")
                    nc.vector.tensor_add(v2[:], v[:], u2[:])
                    yt = ypool.tile([128, U], fp16, tag=f"y{g}")
                    nc.vector.tensor_add(yt[:], v2[:], yd[:])
                    ycl[(g, h)] = yt

                    # interleave previous quarter's transpose+store with
                    # this quarter's conv so PE never sits at a barrier
                    if h > 0:
                        out_block(
                            (h - 1) * (U // 128) + g,
                            [ycl[(gg, h - 1)] for gg in range(G)],
                        )

            for lb8 in range(U // 128):
                out_block(
                    (NU - 1) * (U // 128) + lb8,
                    [ycl[(gg, NU - 1)] for gg in range(G)],
                )

    nc.compile()
    return nc


def _get_program():
    if "nc" not in _CACHE:
        _CACHE["nc"] = _build_device_program()
    return _CACHE["nc"]


def kernel(x, weight, bias):
    global LAST_RESULTS
    from concourse import bass_utils

    x = np.asarray(x)
    weight = np.asarray(weight)
    bias = np.asarray(bias)

    nc = _get_program()

    # Host-side prep: per-core batch shard (fp16) + replicated weights.
    w = weight[:, 0, :]  # [C, K]
    w16 = w.astype(np.float16)
    wd4 = np.zeros((G, K, 128, 128), dtype=np.float16)
    idx = np.arange(128)
    for g in range(G):
        for j in range(K):
            wd4[g, j, idx, idx] = w16[g * 128 : (g + 1) * 128, j]
    wd = np.ascontiguousarray(wd4.transpose(2, 0, 1, 3).reshape(128, G * K * 128))
    # per-partition scalar weights [p, g*K+j] (fp32)
    wv = np.ascontiguousarray(
        w.astype(np.float32).reshape(G, 128, K).transpose(1, 0, 2).reshape(128, G * K)
    )
    bv = np.ascontiguousarray(bias.astype(np.float32).reshape(G, 128).T)
    idm = np.eye(128, dtype=np.float16)

    in_maps = []
    for b in range(B):
        in_maps.append(
            {
                "x16": np.ascontiguousarray(x[b]).astype(np.float16),
                "wd": wd,
                "wv": wv,
                "bv": bv,
                "idm": idm,
            }
        )

    trace = bool(int(os.environ.get("KERNEL_TRACE", "0")))
    res = bass_utils.run_bass_kernel_spmd(
        nc, in_maps, core_ids=list(range(B)), trace=trace
    )
    LAST_RESULTS = res
    _CACHE["last_in_maps"] = in_maps

    out = np.empty((B, L, C), dtype=np.float32)
    for b in range(B):
        out[b] = res.results[b]["y16"].astype(np.float32)
    return out
